# revision 31
# baseline (speedup 1.0000x reference)
"""Trainium2 Bass kernel for nn_MlpwithSOMModule (pairwise-concat MLP + max/mask/sum).

Reference computation (B=8, C=4, T=128, D=64, H=128, G=B*C=32):
  entity  = input[:,:,1] -> [G,T,D];  context = input[:,:,0] -> [G,T,D]
  mask    = (context[:,:,0] != 0)                         [G,T]
  x[g,i,j] = concat(context[g,i], entity[g,j])            [G,T,T,2D]
  for l in 0..5: x = tanh(x @ Ws[l] + bs[l])
  score  = (x @ W_out + b_out)[...,0]                     [G,T,T]
  out[g] = sum_i( max_j(score[g,i,j]) * mask[g,i] )       [G]

Sharding: data-parallel over G across 8 cores (4 groups/core); weights
replicated.  On-chip layout is feature-major ([128 features, pairs]) so every
MLP layer is one stationary-weight matmul.  Layer 0 uses the concat split:
  x0 = ctx_i @ W0[:D] + ent_j @ W0[D:]  ->  A[:,i] + Bb[:,j]
with A, Bb computed once per group as [128,128] matrices.

Matmul operands and activations run in bf16 (PSUM accumulation and all
bias/score/max/sum arithmetic stay fp32); the mask is computed from an fp32
slice of the context so (x != 0) is exact.
"""

import numpy as np
import ml_dtypes

import concourse.bacc as bacc
import concourse.mybir as mybir
import concourse.tile as tile
from concourse.bass_utils import run_bass_kernel_spmd

B, C, T, D = 8, 4, 128, 64
H = 2 * D          # 128
G = B * C          # 32 groups
N_CORES = 8
G_LOC = G // N_CORES   # 4 groups per core
NJ_CHUNK = 16          # j's per chunk
CHUNK = NJ_CHUNK * T   # 2048 pairs per chunk
N_CHUNKS = T // NJ_CHUNK  # 8 chunks per group

F32 = mybir.dt.float32
BF16 = mybir.dt.bfloat16
AF = mybir.ActivationFunctionType
ALU = mybir.AluOpType
AX = mybir.AxisListType

# tanh(x) ~= clip(x*E*(t^2+A1*t+B1)*(t^2+A2*t+B2), -1, 1), t=x^2 -- a deg-9
# odd minimax fit on [-3,3] (max err 5.4e-3; runs off above +1/below -1
# outside so the clip is exact there).  Evaluated in two fused custom DVE
# ops so part of the tanh load moves off the (saturated) Scalar engine.
PA1, PB1 = -21.916163674629964, 136.55212619042595
PA2, PB2 = -3.6690720803481134, 36.960418547287524
PE_ = 0.00019335252160606277
N_POLY2 = 10   # chunks of layer 2 also routed to the DVE poly

_cached_nc = None


def _register_poly_ops():
    """Register the two tanh-poly custom DVE ops (idempotent).
      op A: u = x*C2 * (sq(sq(x)) + sq(x)*C0 + C1)
      op B: y = clip(u * (sq(sq(x)) + sq(x)*C0 + C1), -1, 1), x via Src1
    """
    import concourse.dve_ops as DO
    from concourse.dve_spec import Spec, Src0, Src1, C0, C1, C2, Zero, One, \
        sq, maxx, minn, lower
    from concourse.dve_uop import DveOpSpec
    from concourse.dve_table_gen import dve_ver_for
    from concourse.dve_ops import has_src1

    if "TANH_POLY_A" in DO._SUB_OPCODE_FOR_NAME:
        return DO._POLY_OPS

    tA = sq(Src0)
    bodyA = (Src0 * C2) * (sq(tA) + tA * C0 + C1)
    tB = sq(Src1)
    bodyB = minn(maxx(Src0 * (sq(tB) + tB * C0 + C1), Zero - One), One)

    ver = dve_ver_for("TRN2")
    ops = []
    for name, body in (("TANH_POLY_A", bodyA), ("TANH_POLY_B", bodyB)):
        spec = Spec(body=body)
        row = DO._CUSTOM_DVE_ROW_BASE + len(DO.OPS)
        tmp = DveOpSpec(name=name, opcode=row, uops=lower(spec, ver=ver),
                        rd1_en=has_src1(spec))
        op = DO.DveOp(name, spec, subdim=False, uops_sha={ver: tmp.sha(ver)})
        DO.OPS.append(op)
        DO._SUB_OPCODE_FOR_NAME[name] = row
        DO.CUSTOM_DVE_SPECS[name] = spec
        ops.append(op)
    DO._POLY_OPS = tuple(ops)
    return DO._POLY_OPS


def _build_program():
    opA, opB = _register_poly_ops()
    nc = bacc.Bacc("TRN2", target_bir_lowering=False, debug=False,
                   num_devices=N_CORES)

    ctxT_d = nc.dram_tensor("ctxT", [G_LOC, D, T], BF16, kind="ExternalInput")
    entT_d = nc.dram_tensor("entT", [G_LOC, D, T], BF16, kind="ExternalInput")
    ctx0_d = nc.dram_tensor("ctx0", [G_LOC, T, 1], F32, kind="ExternalInput")
    ws_d = nc.dram_tensor("Ws", [6, H, H], BF16, kind="ExternalInput")
    w0b_d = nc.dram_tensor("w0b", [D, H], BF16, kind="ExternalInput")
    bsT_d = nc.dram_tensor("bsT", [H, 6], F32, kind="ExternalInput")
    bsrow_d = nc.dram_tensor("bsrow", [1, 6 * H], BF16, kind="ExternalInput")
    wout_d = nc.dram_tensor("wout", [H, 1], BF16, kind="ExternalInput")
    bout_d = nc.dram_tensor("bout", [T, 1], F32, kind="ExternalInput")
    out_d = nc.dram_tensor("out", [1, G_LOC], F32, kind="ExternalOutput")

    with tile.TileContext(nc) as tc:
        with (
            tc.tile_pool(name="consts", bufs=1) as consts,
            tc.tile_pool(name="zpool", bufs=4) as zpool,
            tc.tile_pool(name="hpool", bufs=6) as hpool,
            tc.tile_pool(name="upool", bufs=3) as upool,
            tc.tile_pool(name="small", bufs=4) as small,
            tc.tile_pool(name="psum", bufs=2, space="PSUM") as psum,
        ):
            # dummy activation first: pulls the tanh ACT_TABLE_LOAD (~2.7us)
            # off the critical path, overlapping it with setup DMAs
            scratch_sb = consts.tile([1, 1], F32)
            scratch2_sb = consts.tile([1, 1], F32)
            nc.gpsimd.memset(scratch_sb[:], 0.0)
            nc.scalar.activation(scratch2_sb[:], scratch_sb[:], AF.Tanh)

            ws_sb = consts.tile([H, 6 * H], BF16)
            w0b_sb = consts.tile([D, H], BF16)
            bsT_sb = consts.tile([H, 6], F32)
            # layer-0 prerequisites first so chunk 0 can start ASAP
            nc.sync.dma_start(ws_sb[:, 0:H], ws_d[0])
            nc.sync.dma_start(w0b_sb[:], w0b_d[:])
            nc.sync.dma_start(bsT_sb[:], bsT_d[:])
            wout_sb = consts.tile([H, 1], BF16)
            bout_sb = consts.tile([T, 1], F32)
            ones_sb = consts.tile([T, 1], F32)
            res_sb = consts.tile([1, G_LOC], F32)
            bsrow_sb = consts.tile([1, 6 * H], BF16)
            nc.sync.dma_start(bsrow_sb[:], bsrow_d[:])
            ones512_sb = consts.tile([1, 512], BF16)
            nc.vector.memset(ones512_sb[:], 1.0)

            # Per-group setup: A/Bb first-layer matrices, mask sources.
            a_sbs, bb_sbs, ctx0_sbs, rmax_sbs = [None] * G_LOC, \
                [None] * G_LOC, [None] * G_LOC, [None] * G_LOC

            def setup_group(g):
                ctxT_sb = consts.tile([D, T], BF16, tag=f"ctx{g}")
                entT_sb = consts.tile([D, T], BF16, tag=f"ent{g}")
                ctx0_sb = consts.tile([T, 1], F32, tag=f"ctx0_{g}")
                nc.sync.dma_start(ctxT_sb[:], ctxT_d[g])
                nc.sync.dma_start(entT_sb[:], entT_d[g])
                nc.sync.dma_start(ctx0_sb[:], ctx0_d[g])
                # A = (ctx @ W0_top).T : [H, T(i)];  Bb = (ent @ W0_bot).T + b0
                ps_ab = psum.tile([H, CHUNK], F32, tag="mm")
                nc.tensor.matmul(ps_ab[:, 0:T], ws_sb[0:D, 0:H],
                                 ctxT_sb[:], start=True, stop=True)
                nc.tensor.matmul(ps_ab[:, 512:512 + T], w0b_sb[:],
                                 entT_sb[:], start=True, stop=True)
                a_sb = consts.tile([H, T], BF16, tag=f"a{g}")
                nc.vector.tensor_copy(a_sb[:], ps_ab[:, 0:T])
                bb_sb = consts.tile([H, T], F32, tag=f"bb{g}")
                nc.vector.tensor_scalar_add(bb_sb[:], ps_ab[:, 512:512 + T],
                                            bsT_sb[:, 0:1])
                rmax_sb = consts.tile([T, 1], F32, tag=f"rmax{g}")
                nc.vector.memset(rmax_sb[:], -1e30)
                a_sbs[g], bb_sbs[g] = a_sb, bb_sb
                ctx0_sbs[g], rmax_sbs[g] = ctx0_sb, rmax_sb

            def build_z(cc):
                """Layer 0 for global chunk cc: z[:, jl*T+i] = A[:,i]+Bb[:,j]."""
                g = cc // N_CHUNKS
                c = cc % N_CHUNKS
                z_sb = zpool.tile([H, CHUNK], BF16, tag="z")
                for jl in range(NJ_CHUNK):
                    j = c * NJ_CHUNK + jl
                    nc.vector.tensor_scalar_add(
                        z_sb[:, jl * T:(jl + 1) * T], a_sbs[g][:],
                        bb_sbs[g][:, j:j + 1])
                return z_sb

            # group 0 + first z tiles go first so ACT can start ASAP;
            # the rest of the setup DMAs overlap with the first chunks
            setup_group(0)
            z_tiles = {0: build_z(0), 1: build_z(1)}
            for l in range(1, 6):
                nc.sync.dma_start(ws_sb[:, l * H:(l + 1) * H], ws_d[l])
            nc.sync.dma_start(wout_sb[:], wout_d[:])
            nc.sync.dma_start(bout_sb[:], bout_d[:])
            nc.vector.memset(ones_sb[:], 1.0)
            for g in range(1, G_LOC):
                setup_group(g)

            TOT = G_LOC * N_CHUNKS  # 32 chunks, processed in pairs

            for p in range(TOT // 2):
                cA, cB = 2 * p, 2 * p + 1
                h_cur = {}
                for cc in (cA, cB):
                    h_sb = hpool.tile([H, CHUNK], BF16, tag="h")
                    nc.scalar.activation(h_sb[:], z_tiles.pop(cc)[:], AF.Tanh)
                    h_cur[cc] = (h_sb, 0)
                # prefetch next pair's layer-0 on DVE (ahead of the reduces)
                if cB + 2 < TOT:
                    z_tiles[cA + 2] = build_z(cA + 2)
                    z_tiles[cB + 2] = build_z(cB + 2)

                for l in range(1, 6):
                    for cc in (cA, cB):
                        use_poly = l == 1 or (l == 2 and cc < N_POLY2)
                        ht, hoff = h_cur[cc]
                        ps = psum.tile([H, CHUNK], F32, tag="mm")
                        for q in range(4):
                            sl = slice(q * 512, (q + 1) * 512)
                            if use_poly:
                                # fold the bias in on the PE (rank-1 matmul)
                                # since the DVE poly has no bias slot
                                nc.tensor.matmul(
                                    ps[:, sl],
                                    bsrow_sb[0:1, l * H:(l + 1) * H],
                                    ones512_sb[:], start=True, stop=False)
                            nc.tensor.matmul(
                                ps[:, sl],
                                ws_sb[:, l * H:(l + 1) * H],
                                ht[:, hoff + q * 512:hoff + (q + 1) * 512],
                                start=not use_poly, stop=True)
                        h2_sb = hpool.tile([H, CHUNK], BF16, tag="h")
                        if use_poly:
                            u_sb = upool.tile([H, CHUNK], F32, tag="u")
                            nc.vector._custom_dve(opA, out=u_sb[:], in0=ps[:],
                                                  s0=PA1, s1=PB1, imm2=PE_)
                            nc.vector._custom_dve(opB, out=h2_sb[:],
                                                  in0=u_sb[:], in1=ps[:],
                                                  s0=PA2, s1=PB2)
                        else:
                            nc.scalar.activation(h2_sb[:], ps[:], AF.Tanh,
                                                 bias=bsT_sb[:, l:l + 1])
                        h_cur[cc] = (h2_sb, 0)

                for cc in (cA, cB):
                    g = cc // N_CHUNKS
                    ht, hoff = h_cur[cc]
                    # final layer, transposed: score col [T(i), 1] per j via
                    # stationary h-block x moving W_out
                    sc_ps = psum.tile([H, CHUNK], F32, tag="mm")
                    for jl in range(NJ_CHUNK):
                        nc.tensor.matmul(
                            sc_ps[:, jl:jl + 1],
                            ht[:, hoff + jl * T:hoff + (jl + 1) * T],
                            wout_sb[:], start=True, stop=True)
                    # max over the 16 j's, fold into running max (both [T,1])
                    tmp_sb = small.tile([T, 1], F32, tag="tmp")
                    nc.vector.tensor_reduce(tmp_sb[:], sc_ps[:, 0:NJ_CHUNK],
                                            axis=AX.X, op=ALU.max)
                    nc.vector.tensor_max(rmax_sbs[g][:], rmax_sbs[g][:],
                                         tmp_sb[:])

            for g in range(G_LOC):
                # mask = (ctx[:,0] != 0); out = sum_i(mask*(rmax+b_out))
                mask_sb = small.tile([T, 1], F32, tag="mask")
                nc.vector.tensor_scalar(mask_sb[:], ctx0_sbs[g][:], 0.0, None,
                                        op0=ALU.not_equal)
                rb_sb = small.tile([T, 1], F32, tag="rb")
                nc.vector.tensor_scalar_add(rb_sb[:], rmax_sbs[g][:],
                                            bout_sb[:, 0:1])
                mm_sb = small.tile([T, 1], F32, tag="mmul")
                nc.vector.tensor_mul(mm_sb[:], rb_sb[:], mask_sb[:])
                # partition-axis sum via ones-matmul: [1,1] = mm.T @ ones
                sum_ps = psum.tile([H, CHUNK], F32, tag="mm")
                nc.tensor.matmul(sum_ps[0:1, 0:1], mm_sb[:], ones_sb[:],
                                 start=True, stop=True)
                nc.vector.tensor_copy(res_sb[0:1, g:g + 1], sum_ps[0:1, 0:1])

            nc.sync.dma_start(out_d[:], res_sb[:])

    nc.compile()
    return nc


def _get_nc():
    global _cached_nc
    if _cached_nc is None:
        _cached_nc = _build_program()
    return _cached_nc


def _bf16(a):
    return np.ascontiguousarray(a.astype(ml_dtypes.bfloat16))


def _prep_in_maps(input, Ws, bs, W_out, b_out):
    input = np.ascontiguousarray(np.asarray(input, dtype=np.float32))
    Ws = np.asarray(Ws, dtype=np.float32)
    bs = np.asarray(bs, dtype=np.float32)
    W_out = np.asarray(W_out, dtype=np.float32)
    b_out = np.asarray(b_out, dtype=np.float32)

    ctx = input[:, :, 0].reshape(G, T, D)
    ent = input[:, :, 1].reshape(G, T, D)
    ctxT = _bf16(ctx.transpose(0, 2, 1))                  # [G, D, T]
    entT = _bf16(ent.transpose(0, 2, 1))
    ctx0 = np.ascontiguousarray(ctx[:, :, 0]).reshape(G, T, 1)  # fp32
    ws_bf = _bf16(Ws)
    w0b = _bf16(Ws[0][D:H])
    bsT = np.ascontiguousarray(bs.T)                      # [H, 6]
    bsrow = _bf16(bs.reshape(1, 6 * H))
    wout = _bf16(W_out)
    bout = np.broadcast_to(b_out.reshape(1, 1), (T, 1)).copy()

    in_maps = []
    for k in range(N_CORES):
        sl = slice(k * G_LOC, (k + 1) * G_LOC)
        in_maps.append({
            "ctxT": np.ascontiguousarray(ctxT[sl]),
            "entT": np.ascontiguousarray(entT[sl]),
            "ctx0": np.ascontiguousarray(ctx0[sl]),
            "Ws": ws_bf,
            "w0b": w0b,
            "bsT": bsT,
            "bsrow": bsrow,
            "wout": wout,
            "bout": bout,
        })
    return in_maps


def run_traced(trace=False, **inputs):
    """Returns (output [G], exec_time_ns or None)."""
    nc = _get_nc()
    in_maps = _prep_in_maps(**inputs)
    res = run_bass_kernel_spmd(nc, in_maps, list(range(N_CORES)), trace=trace)
    out = np.concatenate([res.results[k]["out"].reshape(G_LOC)
                          for k in range(N_CORES)])
    return out, res.exec_time_ns


def kernel(**inputs) -> np.ndarray:
    out, _ = run_traced(trace=False, **inputs)
    return out


# revision 38
# speedup vs baseline: 1.1270x; 1.1270x over previous
"""Trainium2 Bass kernel for nn_MlpwithSOMModule (pairwise-concat MLP + max/mask/sum).

Reference computation (B=8, C=4, T=128, D=64, H=128, G=B*C=32):
  entity  = input[:,:,1] -> [G,T,D];  context = input[:,:,0] -> [G,T,D]
  mask    = (context[:,:,0] != 0)                         [G,T]
  x[g,i,j] = concat(context[g,i], entity[g,j])            [G,T,T,2D]
  for l in 0..5: x = tanh(x @ Ws[l] + bs[l])
  score  = (x @ W_out + b_out)[...,0]                     [G,T,T]
  out[g] = sum_i( max_j(score[g,i,j]) * mask[g,i] )       [G]

Sharding: data-parallel over G across 8 cores (4 groups/core); weights
replicated.  On-chip layout is feature-major ([128 features, pairs]) so every
MLP layer is one stationary-weight matmul.  Layer 0 uses the concat split:
  x0 = ctx_i @ W0[:D] + ent_j @ W0[D:]  ->  A[:,i] + Bb[:,j]
with A, Bb computed once per group as [128,128] matrices.

Matmul operands and activations run in bf16 (PSUM accumulation and all
bias/score/max/sum arithmetic stay fp32); the mask is computed from an fp32
slice of the context so (x != 0) is exact.
"""

import numpy as np
import ml_dtypes

import concourse.bacc as bacc
import concourse.mybir as mybir
import concourse.tile as tile
from concourse.bass_utils import run_bass_kernel_spmd

B, C, T, D = 8, 4, 128, 64
H = 2 * D          # 128
G = B * C          # 32 groups
N_CORES = 8
G_LOC = G // N_CORES   # 4 groups per core
NJ_CHUNK = 16          # j's per chunk
CHUNK = NJ_CHUNK * T   # 2048 pairs per chunk
N_CHUNKS = T // NJ_CHUNK  # 8 chunks per group

F32 = mybir.dt.float32
BF16 = mybir.dt.bfloat16
AF = mybir.ActivationFunctionType
ALU = mybir.AluOpType
AX = mybir.AxisListType

# tanh(x) ~= clip(x*E*(t^2+A1*t+B1)*(t^2+A2*t+B2), -1, 1), t=x^2 -- a deg-9
# odd minimax fit on [-3,3] (max err 5.4e-3; runs off above +1/below -1
# outside so the clip is exact there).  Evaluated in two fused custom DVE
# ops so part of the tanh load moves off the (saturated) Scalar engine.
PA1, PB1 = -21.916163674629964, 136.55212619042595
PA2, PB2 = -3.6690720803481134, 36.960418547287524
PE_ = 0.00019335252160606277
N_POLY2 = 16   # chunks of layer 2 also routed to the DVE poly

_cached_nc = {}


def _register_poly_ops():
    """Register the two tanh-poly custom DVE ops (idempotent).
      op A: u = x*C2 * (sq(sq(x)) + sq(x)*C0 + C1)
      op B: y = clip(u * (sq(sq(x)) + sq(x)*C0 + C1), -1, 1), x via Src1
    """
    import concourse.dve_ops as DO
    from concourse.dve_spec import Spec, Src0, Src1, C0, C1, C2, Zero, One, \
        sq, maxx, minn, lower
    from concourse.dve_uop import DveOpSpec
    from concourse.dve_table_gen import dve_ver_for
    from concourse.dve_ops import has_src1

    if "TANH_POLY_A" in DO._SUB_OPCODE_FOR_NAME:
        return DO._POLY_OPS

    tA = sq(Src0)
    bodyA = (Src0 * C2) * (sq(tA) + tA * C0 + C1)
    tB = sq(Src1)
    bodyB = minn(maxx(Src0 * (sq(tB) + tB * C0 + C1), Zero - One), One)

    ver = dve_ver_for("TRN2")
    ops = []
    for name, body in (("TANH_POLY_A", bodyA), ("TANH_POLY_B", bodyB)):
        spec = Spec(body=body)
        row = DO._CUSTOM_DVE_ROW_BASE + len(DO.OPS)
        tmp = DveOpSpec(name=name, opcode=row, uops=lower(spec, ver=ver),
                        rd1_en=has_src1(spec))
        op = DO.DveOp(name, spec, subdim=False, uops_sha={ver: tmp.sha(ver)})
        DO.OPS.append(op)
        DO._SUB_OPCODE_FOR_NAME[name] = row
        DO.CUSTOM_DVE_SPECS[name] = spec
        ops.append(op)
    DO._POLY_OPS = tuple(ops)
    return DO._POLY_OPS


def _build_program(bias_zero):
    opA, opB = _register_poly_ops()
    nc = bacc.Bacc("TRN2", target_bir_lowering=False, debug=False,
                   num_devices=N_CORES)

    ctxT_d = nc.dram_tensor("ctxT", [G_LOC, D, T], BF16, kind="ExternalInput")
    entT_d = nc.dram_tensor("entT", [G_LOC, D, T], BF16, kind="ExternalInput")
    ctx0_d = nc.dram_tensor("ctx0", [G_LOC, T, 1], F32, kind="ExternalInput")
    ws_d = nc.dram_tensor("Ws", [6, H, H], BF16, kind="ExternalInput")
    w0b_d = nc.dram_tensor("w0b", [D, H], BF16, kind="ExternalInput")
    bsT_d = nc.dram_tensor("bsT", [H, 6], F32, kind="ExternalInput")
    bsrow_d = nc.dram_tensor("bsrow", [1, 6 * H], BF16, kind="ExternalInput")
    wout_d = nc.dram_tensor("wout", [H, 1], BF16, kind="ExternalInput")
    bout_d = nc.dram_tensor("bout", [T, 1], F32, kind="ExternalInput")
    out_d = nc.dram_tensor("out", [1, G_LOC], F32, kind="ExternalOutput")

    with tile.TileContext(nc) as tc:
        with (
            tc.tile_pool(name="consts", bufs=1) as consts,
            tc.tile_pool(name="zpool", bufs=4) as zpool,
            tc.tile_pool(name="hpool", bufs=6) as hpool,
            tc.tile_pool(name="upool", bufs=3) as upool,
            tc.tile_pool(name="small", bufs=4) as small,
            tc.tile_pool(name="psum", bufs=2, space="PSUM") as psum,
        ):
            # dummy activation first: pulls the tanh ACT_TABLE_LOAD (~2.7us)
            # off the critical path, overlapping it with setup DMAs
            scratch_sb = consts.tile([1, 1], F32)
            scratch2_sb = consts.tile([1, 1], F32)
            nc.gpsimd.memset(scratch_sb[:], 0.0)
            nc.scalar.activation(scratch2_sb[:], scratch_sb[:], AF.Tanh)

            ws_sb = consts.tile([H, 6 * H], BF16)
            w0b_sb = consts.tile([D, H], BF16)
            bsT_sb = consts.tile([H, 6], F32)
            # layer-0 prerequisites first so chunk 0 can start ASAP
            nc.sync.dma_start(ws_sb[:, 0:H], ws_d[0])
            nc.sync.dma_start(w0b_sb[:], w0b_d[:])
            nc.sync.dma_start(bsT_sb[:], bsT_d[:])
            wout_sb = consts.tile([H, 1], BF16)
            bout_sb = consts.tile([T, 1], F32)
            ones_sb = consts.tile([T, 1], F32)
            res_sb = consts.tile([1, G_LOC], F32)
            bsrow_sb = consts.tile([1, 6 * H], BF16)
            nc.sync.dma_start(bsrow_sb[:], bsrow_d[:])
            ones512_sb = consts.tile([1, 512], BF16)
            nc.vector.memset(ones512_sb[:], 1.0)

            # Per-group setup: A/Bb first-layer matrices, mask sources.
            a_sbs, bb_sbs, ctx0_sbs, rmax_sbs = [None] * G_LOC, \
                [None] * G_LOC, [None] * G_LOC, [None] * G_LOC

            def setup_group(g):
                ctxT_sb = consts.tile([D, T], BF16, tag=f"ctx{g}")
                entT_sb = consts.tile([D, T], BF16, tag=f"ent{g}")
                ctx0_sb = consts.tile([T, 1], F32, tag=f"ctx0_{g}")
                nc.sync.dma_start(ctxT_sb[:], ctxT_d[g])
                nc.sync.dma_start(entT_sb[:], entT_d[g])
                nc.sync.dma_start(ctx0_sb[:], ctx0_d[g])
                # A = (ctx @ W0_top).T : [H, T(i)];  Bb = (ent @ W0_bot).T + b0
                ps_ab = psum.tile([H, CHUNK], F32, tag="mm")
                nc.tensor.matmul(ps_ab[:, 0:T], ws_sb[0:D, 0:H],
                                 ctxT_sb[:], start=True, stop=True)
                nc.tensor.matmul(ps_ab[:, 512:512 + T], w0b_sb[:],
                                 entT_sb[:], start=True, stop=True)
                a_sb = consts.tile([H, T], BF16, tag=f"a{g}")
                nc.vector.tensor_copy(a_sb[:], ps_ab[:, 0:T])
                bb_sb = consts.tile([H, T], BF16, tag=f"bb{g}")
                nc.vector.tensor_scalar_add(bb_sb[:], ps_ab[:, 512:512 + T],
                                            bsT_sb[:, 0:1])
                rmax_sb = consts.tile([T, 1], F32, tag=f"rmax{g}")
                nc.vector.memset(rmax_sb[:], -1e30)
                a_sbs[g], bb_sbs[g] = a_sb, bb_sb
                ctx0_sbs[g], rmax_sbs[g] = ctx0_sb, rmax_sb

            def build_z(cc):
                """Layer 0 for global chunk cc: z[:, jl*T+i] = A[:,i]+Bb[:,j].
                One broadcast-AP tensor_tensor on the (otherwise idle) GpSimd
                engine, keeping the Vector engine free for the tanh polys."""
                g = cc // N_CHUNKS
                c = cc % N_CHUNKS
                z_sb = zpool.tile([H, CHUNK], BF16, tag="z")
                from concourse.bass import broadcast_tensor_aps
                a_ap = a_sbs[g][:].rearrange("p (o i) -> p o i", o=1)
                b_ap = bb_sbs[g][:, c * NJ_CHUNK:(c + 1) * NJ_CHUNK] \
                    .rearrange("p (j o) -> p j o", o=1)
                a_b, b_b = broadcast_tensor_aps(a_ap, b_ap)
                z_ap = z_sb[:].rearrange("p (j i) -> p j i", j=NJ_CHUNK)
                nc.gpsimd.tensor_tensor(z_ap, a_b, b_b, op=ALU.add)
                return z_sb

            # group 0 + first z tiles go first so ACT can start ASAP;
            # the rest of the setup DMAs overlap with the first chunks
            setup_group(0)
            z_tiles = {0: build_z(0), 1: build_z(1)}
            for l in range(1, 6):
                nc.sync.dma_start(ws_sb[:, l * H:(l + 1) * H], ws_d[l])
            nc.sync.dma_start(wout_sb[:], wout_d[:])
            nc.sync.dma_start(bout_sb[:], bout_d[:])
            nc.vector.memset(ones_sb[:], 1.0)
            for g in range(1, G_LOC):
                setup_group(g)

            TOT = G_LOC * N_CHUNKS  # 32 chunks, processed in pairs

            for p in range(TOT // 2):
                cA, cB = 2 * p, 2 * p + 1
                h_cur = {}
                for cc in (cA, cB):
                    h_sb = hpool.tile([H, CHUNK], BF16, tag="h")
                    nc.scalar.activation(h_sb[:], z_tiles.pop(cc)[:], AF.Tanh)
                    h_cur[cc] = (h_sb, 0)
                # prefetch next pair's layer-0 on DVE (ahead of the reduces)
                if cB + 2 < TOT:
                    z_tiles[cA + 2] = build_z(cA + 2)
                    z_tiles[cB + 2] = build_z(cB + 2)

                for l in range(1, 6):
                    for cc in (cA, cB):
                        use_poly = l == 1 or (l == 2 and cc < N_POLY2)
                        need_bias_mm = use_poly and not bias_zero
                        ht, hoff = h_cur[cc]
                        ps = psum.tile([H, CHUNK], F32, tag="mm")
                        if need_bias_mm:
                            # fold the bias in on the PE (rank-1 matmuls,
                            # grouped so the stationary loads only once)
                            # since the DVE poly has no bias slot
                            for q in range(4):
                                nc.tensor.matmul(
                                    ps[:, q * 512:(q + 1) * 512],
                                    bsrow_sb[0:1, l * H:(l + 1) * H],
                                    ones512_sb[:], start=True, stop=False)
                        for q in range(4):
                            nc.tensor.matmul(
                                ps[:, q * 512:(q + 1) * 512],
                                ws_sb[:, l * H:(l + 1) * H],
                                ht[:, hoff + q * 512:hoff + (q + 1) * 512],
                                start=not need_bias_mm, stop=True)
                        h2_sb = hpool.tile([H, CHUNK], BF16, tag="h")
                        if use_poly:
                            u_sb = upool.tile([H, CHUNK], F32, tag="u")
                            nc.vector._custom_dve(opA, out=u_sb[:], in0=ps[:],
                                                  s0=PA1, s1=PB1, imm2=PE_)
                            nc.vector._custom_dve(opB, out=h2_sb[:],
                                                  in0=u_sb[:], in1=ps[:],
                                                  s0=PA2, s1=PB2)
                        else:
                            nc.scalar.activation(h2_sb[:], ps[:], AF.Tanh,
                                                 bias=bsT_sb[:, l:l + 1])
                        h_cur[cc] = (h2_sb, 0)

                for cc in (cA, cB):
                    g = cc // N_CHUNKS
                    ht, hoff = h_cur[cc]
                    # final layer, transposed: score col [T(i), 1] per j via
                    # stationary h-block x moving W_out
                    sc_ps = psum.tile([H, CHUNK], F32, tag="mm")
                    for jl in range(NJ_CHUNK):
                        nc.tensor.matmul(
                            sc_ps[:, jl:jl + 1],
                            ht[:, hoff + jl * T:hoff + (jl + 1) * T],
                            wout_sb[:], start=True, stop=True)
                    # max over the 16 j's, fold into running max (both [T,1])
                    tmp_sb = small.tile([T, 1], F32, tag="tmp")
                    nc.vector.tensor_reduce(tmp_sb[:], sc_ps[:, 0:NJ_CHUNK],
                                            axis=AX.X, op=ALU.max)
                    nc.vector.tensor_max(rmax_sbs[g][:], rmax_sbs[g][:],
                                         tmp_sb[:])

            for g in range(G_LOC):
                # mask = (ctx[:,0] != 0); out = sum_i(mask*(rmax+b_out))
                mask_sb = small.tile([T, 1], F32, tag="mask")
                nc.vector.tensor_scalar(mask_sb[:], ctx0_sbs[g][:], 0.0, None,
                                        op0=ALU.not_equal)
                rb_sb = small.tile([T, 1], F32, tag="rb")
                nc.vector.tensor_scalar_add(rb_sb[:], rmax_sbs[g][:],
                                            bout_sb[:, 0:1])
                mm_sb = small.tile([T, 1], F32, tag="mmul")
                nc.vector.tensor_mul(mm_sb[:], rb_sb[:], mask_sb[:])
                # partition-axis sum via ones-matmul: [1,1] = mm.T @ ones
                sum_ps = psum.tile([H, CHUNK], F32, tag="mm")
                nc.tensor.matmul(sum_ps[0:1, 0:1], mm_sb[:], ones_sb[:],
                                 start=True, stop=True)
                nc.vector.tensor_copy(res_sb[0:1, g:g + 1], sum_ps[0:1, 0:1])

            nc.sync.dma_start(out_d[:], res_sb[:])

    nc.compile()
    return nc


def _get_nc(bias_zero):
    if bias_zero not in _cached_nc:
        _cached_nc[bias_zero] = _build_program(bias_zero)
    return _cached_nc[bias_zero]


def _bf16(a):
    return np.ascontiguousarray(a.astype(ml_dtypes.bfloat16))


def _prep_in_maps(input, Ws, bs, W_out, b_out):
    input = np.ascontiguousarray(np.asarray(input, dtype=np.float32))
    Ws = np.asarray(Ws, dtype=np.float32)
    bs = np.asarray(bs, dtype=np.float32)
    W_out = np.asarray(W_out, dtype=np.float32)
    b_out = np.asarray(b_out, dtype=np.float32)

    ctx = input[:, :, 0].reshape(G, T, D)
    ent = input[:, :, 1].reshape(G, T, D)
    ctxT = _bf16(ctx.transpose(0, 2, 1))                  # [G, D, T]
    entT = _bf16(ent.transpose(0, 2, 1))
    ctx0 = np.ascontiguousarray(ctx[:, :, 0]).reshape(G, T, 1)  # fp32
    ws_bf = _bf16(Ws)
    w0b = _bf16(Ws[0][D:H])
    bsT = np.ascontiguousarray(bs.T)                      # [H, 6]
    bsrow = _bf16(bs.reshape(1, 6 * H))
    wout = _bf16(W_out)
    bout = np.broadcast_to(b_out.reshape(1, 1), (T, 1)).copy()

    in_maps = []
    for k in range(N_CORES):
        sl = slice(k * G_LOC, (k + 1) * G_LOC)
        in_maps.append({
            "ctxT": np.ascontiguousarray(ctxT[sl]),
            "entT": np.ascontiguousarray(entT[sl]),
            "ctx0": np.ascontiguousarray(ctx0[sl]),
            "Ws": ws_bf,
            "w0b": w0b,
            "bsT": bsT,
            "bsrow": bsrow,
            "wout": wout,
            "bout": bout,
        })
    return in_maps


def run_traced(trace=False, **inputs):
    """Returns (output [G], exec_time_ns or None)."""
    nc = _get_nc(bias_zero=bool(np.all(np.asarray(inputs["bs"]) == 0)))
    in_maps = _prep_in_maps(**inputs)
    res = run_bass_kernel_spmd(nc, in_maps, list(range(N_CORES)), trace=trace)
    out = np.concatenate([res.results[k]["out"].reshape(G_LOC)
                          for k in range(N_CORES)])
    return out, res.exec_time_ns


def kernel(**inputs) -> np.ndarray:
    out, _ = run_traced(trace=False, **inputs)
    return out


# revision 43
# speedup vs baseline: 1.1789x; 1.0461x over previous
"""Trainium2 Bass kernel for nn_MlpwithSOMModule (pairwise-concat MLP + max/mask/sum).

Reference computation (B=8, C=4, T=128, D=64, H=128, G=B*C=32):
  entity  = input[:,:,1] -> [G,T,D];  context = input[:,:,0] -> [G,T,D]
  mask    = (context[:,:,0] != 0)                         [G,T]
  x[g,i,j] = concat(context[g,i], entity[g,j])            [G,T,T,2D]
  for l in 0..5: x = tanh(x @ Ws[l] + bs[l])
  score  = (x @ W_out + b_out)[...,0]                     [G,T,T]
  out[g] = sum_i( max_j(score[g,i,j]) * mask[g,i] )       [G]

Sharding: data-parallel over G across 8 cores (4 groups/core); weights
replicated.  On-chip layout is feature-major ([128 features, pairs]) so every
MLP layer is one stationary-weight matmul.  Layer 0 uses the concat split:
  x0 = ctx_i @ W0[:D] + ent_j @ W0[D:]  ->  A[:,i] + Bb[:,j]
with A, Bb computed once per group as [128,128] matrices.

Matmul operands and activations run in bf16 (PSUM accumulation and all
bias/score/max/sum arithmetic stay fp32); the mask is computed from an fp32
slice of the context so (x != 0) is exact.
"""

import numpy as np
import ml_dtypes

import concourse.bacc as bacc
import concourse.mybir as mybir
import concourse.tile as tile
from concourse.bass_utils import run_bass_kernel_spmd

B, C, T, D = 8, 4, 128, 64
H = 2 * D          # 128
G = B * C          # 32 groups
N_CORES = 8
G_LOC = G // N_CORES   # 4 groups per core
NJ_CHUNK = 16          # j's per chunk
CHUNK = NJ_CHUNK * T   # 2048 pairs per chunk
N_CHUNKS = T // NJ_CHUNK  # 8 chunks per group

F32 = mybir.dt.float32
BF16 = mybir.dt.bfloat16
AF = mybir.ActivationFunctionType
ALU = mybir.AluOpType
AX = mybir.AxisListType

# tanh(x) ~= clip(x*E*(t^2+A1*t+B1)*(t^2+A2*t+B2), -1, 1), t=x^2 -- a deg-9
# odd minimax fit on [-3,3] (max err 5.4e-3; runs off above +1/below -1
# outside so the clip is exact there).  Evaluated in two fused custom DVE
# ops so part of the tanh load moves off the (saturated) Scalar engine.
PA1, PB1 = -21.916163674629964, 136.55212619042595
PA2, PB2 = -3.6690720803481134, 36.960418547287524
PE_ = 0.00019335252160606277
N_POLY2 = 16   # chunks of layer 2 also routed to the DVE poly

_cached_nc = {}


def _register_poly_ops():
    """Register the two tanh-poly custom DVE ops (idempotent).
      op A: u = x*C2 * (sq(sq(x)) + sq(x)*C0 + C1)
      op B: y = clip(u * (sq(sq(x)) + sq(x)*C0 + C1), -1, 1), x via Src1
    """
    import concourse.dve_ops as DO
    from concourse.dve_spec import Spec, Src0, Src1, C0, C1, C2, Zero, One, \
        sq, maxx, minn, lower
    from concourse.dve_uop import DveOpSpec
    from concourse.dve_table_gen import dve_ver_for
    from concourse.dve_ops import has_src1

    if "TANH_POLY_A" in DO._SUB_OPCODE_FOR_NAME:
        return DO._POLY_OPS

    tA = sq(Src0)
    bodyA = (Src0 * C2) * (sq(tA) + tA * C0 + C1)
    tB = sq(Src1)
    bodyB = minn(maxx(Src0 * (sq(tB) + tB * C0 + C1), Zero - One), One)

    ver = dve_ver_for("TRN2")
    ops = []
    for name, body in (("TANH_POLY_A", bodyA), ("TANH_POLY_B", bodyB)):
        spec = Spec(body=body)
        row = DO._CUSTOM_DVE_ROW_BASE + len(DO.OPS)
        tmp = DveOpSpec(name=name, opcode=row, uops=lower(spec, ver=ver),
                        rd1_en=has_src1(spec))
        op = DO.DveOp(name, spec, subdim=False, uops_sha={ver: tmp.sha(ver)})
        DO.OPS.append(op)
        DO._SUB_OPCODE_FOR_NAME[name] = row
        DO.CUSTOM_DVE_SPECS[name] = spec
        ops.append(op)
    DO._POLY_OPS = tuple(ops)
    return DO._POLY_OPS


def _build_program(bias_zero):
    opA, opB = _register_poly_ops()
    nc = bacc.Bacc("TRN2", target_bir_lowering=False, debug=False,
                   num_devices=N_CORES)

    ctxT_d = nc.dram_tensor("ctxT", [G_LOC, D, T], BF16, kind="ExternalInput")
    entT_d = nc.dram_tensor("entT", [G_LOC, D, T], BF16, kind="ExternalInput")
    ctx0_d = nc.dram_tensor("ctx0", [G_LOC, T, 1], F32, kind="ExternalInput")
    ws_d = nc.dram_tensor("Ws", [6, H, H], BF16, kind="ExternalInput")
    w0b_d = nc.dram_tensor("w0b", [D, H], BF16, kind="ExternalInput")
    bsT_d = nc.dram_tensor("bsT", [H, 6], F32, kind="ExternalInput")
    bsrow_d = nc.dram_tensor("bsrow", [1, 6 * H], BF16, kind="ExternalInput")
    wout_d = nc.dram_tensor("wout", [H, 1], BF16, kind="ExternalInput")
    bout_d = nc.dram_tensor("bout", [T, 1], F32, kind="ExternalInput")
    out_d = nc.dram_tensor("out", [1, G_LOC], F32, kind="ExternalOutput")

    with tile.TileContext(nc) as tc:
        with (
            tc.tile_pool(name="consts", bufs=1) as consts,
            tc.tile_pool(name="zpool", bufs=4) as zpool,
            tc.tile_pool(name="hpool", bufs=6) as hpool,
            tc.tile_pool(name="upool", bufs=4) as upool,
            tc.tile_pool(name="small", bufs=4) as small,
            tc.tile_pool(name="psum", bufs=4, space="PSUM") as psum,
        ):
            # dummy activation first: pulls the tanh ACT_TABLE_LOAD (~2.7us)
            # off the critical path, overlapping it with setup DMAs
            scratch_sb = consts.tile([1, 1], F32)
            scratch2_sb = consts.tile([1, 1], F32)
            nc.gpsimd.memset(scratch_sb[:], 0.0)
            nc.scalar.activation(scratch2_sb[:], scratch_sb[:], AF.Tanh)

            ws_sb = consts.tile([H, 6 * H], BF16)
            w0b_sb = consts.tile([D, H], BF16)
            bsT_sb = consts.tile([H, 6], F32)
            # layer-0 prerequisites first so chunk 0 can start ASAP
            nc.sync.dma_start(ws_sb[:, 0:H], ws_d[0])
            nc.sync.dma_start(w0b_sb[:], w0b_d[:])
            nc.sync.dma_start(bsT_sb[:], bsT_d[:])
            wout_sb = consts.tile([H, 1], BF16)
            bout_sb = consts.tile([T, 1], F32)
            ones_sb = consts.tile([T, 1], F32)
            res_sb = consts.tile([1, G_LOC], F32)
            bsrow_sb = consts.tile([1, 6 * H], BF16)
            nc.sync.dma_start(bsrow_sb[:], bsrow_d[:])
            ones512_sb = consts.tile([1, 512], BF16)
            nc.vector.memset(ones512_sb[:], 1.0)

            # Per-group setup: A/Bb first-layer matrices, mask sources.
            a_sbs, bb_sbs, ctx0_sbs, rmax_sbs = [None] * G_LOC, \
                [None] * G_LOC, [None] * G_LOC, [None] * G_LOC

            def setup_group(g):
                ctxT_sb = consts.tile([D, T], BF16, tag=f"ctx{g}")
                entT_sb = consts.tile([D, T], BF16, tag=f"ent{g}")
                ctx0_sb = consts.tile([T, 1], F32, tag=f"ctx0_{g}")
                nc.sync.dma_start(ctxT_sb[:], ctxT_d[g])
                nc.sync.dma_start(entT_sb[:], entT_d[g])
                nc.sync.dma_start(ctx0_sb[:], ctx0_d[g])
                # A = (ctx @ W0_top).T : [H, T(i)];  Bb = (ent @ W0_bot).T + b0
                ps_ab = psum.tile([H, CHUNK // 2], F32, tag="mm")
                nc.tensor.matmul(ps_ab[:, 0:T], ws_sb[0:D, 0:H],
                                 ctxT_sb[:], start=True, stop=True)
                nc.tensor.matmul(ps_ab[:, 512:512 + T], w0b_sb[:],
                                 entT_sb[:], start=True, stop=True)
                a_sb = consts.tile([H, T], BF16, tag=f"a{g}")
                nc.vector.tensor_copy(a_sb[:], ps_ab[:, 0:T])
                bb_sb = consts.tile([H, T], BF16, tag=f"bb{g}")
                nc.vector.tensor_scalar_add(bb_sb[:], ps_ab[:, 512:512 + T],
                                            bsT_sb[:, 0:1])
                rmax_sb = consts.tile([T, 1], F32, tag=f"rmax{g}")
                nc.vector.memset(rmax_sb[:], -1e30)
                a_sbs[g], bb_sbs[g] = a_sb, bb_sb
                ctx0_sbs[g], rmax_sbs[g] = ctx0_sb, rmax_sb

            def build_z(cc):
                """Layer 0 for global chunk cc: z[:, jl*T+i] = A[:,i]+Bb[:,j].
                One broadcast-AP tensor_tensor on the (otherwise idle) GpSimd
                engine, keeping the Vector engine free for the tanh polys."""
                g = cc // N_CHUNKS
                c = cc % N_CHUNKS
                z_sb = zpool.tile([H, CHUNK], BF16, tag="z")
                from concourse.bass import broadcast_tensor_aps
                a_ap = a_sbs[g][:].rearrange("p (o i) -> p o i", o=1)
                b_ap = bb_sbs[g][:, c * NJ_CHUNK:(c + 1) * NJ_CHUNK] \
                    .rearrange("p (j o) -> p j o", o=1)
                a_b, b_b = broadcast_tensor_aps(a_ap, b_ap)
                z_ap = z_sb[:].rearrange("p (j i) -> p j i", j=NJ_CHUNK)
                nc.gpsimd.tensor_tensor(z_ap, a_b, b_b, op=ALU.add)
                return z_sb

            # group 0 + first z tiles go first so ACT can start ASAP;
            # the rest of the setup DMAs overlap with the first chunks
            setup_group(0)
            z_tiles = {0: build_z(0), 1: build_z(1)}
            for l in range(1, 6):
                nc.sync.dma_start(ws_sb[:, l * H:(l + 1) * H], ws_d[l])
            nc.sync.dma_start(wout_sb[:], wout_d[:])
            nc.sync.dma_start(bout_sb[:], bout_d[:])
            nc.vector.memset(ones_sb[:], 1.0)
            for g in range(1, G_LOC):
                setup_group(g)

            TOT = G_LOC * N_CHUNKS  # 32 chunks, processed in pairs

            for p in range(TOT // 2):
                cA, cB = 2 * p, 2 * p + 1
                h_cur = {}
                for cc in (cA, cB):
                    h_sb = hpool.tile([H, CHUNK], BF16, tag="h")
                    nc.scalar.activation(h_sb[:], z_tiles.pop(cc)[:], AF.Tanh)
                    h_cur[cc] = (h_sb, 0)
                # prefetch next pair's layer-0 on DVE (ahead of the reduces)
                if cB + 2 < TOT:
                    z_tiles[cA + 2] = build_z(cA + 2)
                    z_tiles[cB + 2] = build_z(cB + 2)

                HC = CHUNK // 2  # 1024: psum tiles span 2 banks -> 4 slots
                for l in range(1, 6):
                    for cc in (cA, cB):
                        use_poly = l == 1 or (l == 2 and cc < N_POLY2)
                        need_bias_mm = use_poly and not bias_zero
                        ht, hoff = h_cur[cc]
                        h2_sb = hpool.tile([H, CHUNK], BF16, tag="h")
                        for half in range(2):
                            ps = psum.tile([H, HC], F32, tag="mm")
                            if need_bias_mm:
                                # fold the bias in on the PE (rank-1 matmuls)
                                # since the DVE poly has no bias slot
                                for q in range(2):
                                    nc.tensor.matmul(
                                        ps[:, q * 512:(q + 1) * 512],
                                        bsrow_sb[0:1, l * H:(l + 1) * H],
                                        ones512_sb[:], start=True, stop=False)
                            for q in range(2):
                                o = hoff + half * HC + q * 512
                                nc.tensor.matmul(
                                    ps[:, q * 512:(q + 1) * 512],
                                    ws_sb[:, l * H:(l + 1) * H],
                                    ht[:, o:o + 512],
                                    start=not need_bias_mm, stop=True)
                            hsl = h2_sb[:, half * HC:(half + 1) * HC]
                            if use_poly:
                                u_sb = upool.tile([H, HC], F32, tag="u")
                                nc.vector._custom_dve(
                                    opA, out=u_sb[:], in0=ps[:],
                                    s0=PA1, s1=PB1, imm2=PE_)
                                nc.vector._custom_dve(
                                    opB, out=hsl, in0=u_sb[:], in1=ps[:],
                                    s0=PA2, s1=PB2)
                            else:
                                nc.scalar.activation(hsl, ps[:], AF.Tanh,
                                                     bias=bsT_sb[:, l:l + 1])
                        h_cur[cc] = (h2_sb, 0)

                for cc in (cA, cB):
                    g = cc // N_CHUNKS
                    ht, hoff = h_cur[cc]
                    # final layer, transposed: score col [T(i), 1] per j via
                    # stationary h-block x moving W_out
                    sc_ps = psum.tile([H, CHUNK // 2], F32, tag="mm")
                    for jl in range(NJ_CHUNK):
                        nc.tensor.matmul(
                            sc_ps[:, jl:jl + 1],
                            ht[:, hoff + jl * T:hoff + (jl + 1) * T],
                            wout_sb[:], start=True, stop=True)
                    # max over the 16 j's, fold into running max (both [T,1])
                    tmp_sb = small.tile([T, 1], F32, tag="tmp")
                    nc.vector.tensor_reduce(tmp_sb[:], sc_ps[:, 0:NJ_CHUNK],
                                            axis=AX.X, op=ALU.max)
                    nc.vector.tensor_max(rmax_sbs[g][:], rmax_sbs[g][:],
                                         tmp_sb[:])

            for g in range(G_LOC):
                # mask = (ctx[:,0] != 0); out = sum_i(mask*(rmax+b_out))
                mask_sb = small.tile([T, 1], F32, tag="mask")
                nc.vector.tensor_scalar(mask_sb[:], ctx0_sbs[g][:], 0.0, None,
                                        op0=ALU.not_equal)
                rb_sb = small.tile([T, 1], F32, tag="rb")
                nc.vector.tensor_scalar_add(rb_sb[:], rmax_sbs[g][:],
                                            bout_sb[:, 0:1])
                mm_sb = small.tile([T, 1], F32, tag="mmul")
                nc.vector.tensor_mul(mm_sb[:], rb_sb[:], mask_sb[:])
                # partition-axis sum via ones-matmul: [1,1] = mm.T @ ones
                sum_ps = psum.tile([H, CHUNK // 2], F32, tag="mm")
                nc.tensor.matmul(sum_ps[0:1, 0:1], mm_sb[:], ones_sb[:],
                                 start=True, stop=True)
                nc.vector.tensor_copy(res_sb[0:1, g:g + 1], sum_ps[0:1, 0:1])

            nc.sync.dma_start(out_d[:], res_sb[:])

    nc.compile()
    return nc


def _get_nc(bias_zero):
    if bias_zero not in _cached_nc:
        _cached_nc[bias_zero] = _build_program(bias_zero)
    return _cached_nc[bias_zero]


def _bf16(a):
    return np.ascontiguousarray(a.astype(ml_dtypes.bfloat16))


def _prep_in_maps(input, Ws, bs, W_out, b_out):
    input = np.ascontiguousarray(np.asarray(input, dtype=np.float32))
    Ws = np.asarray(Ws, dtype=np.float32)
    bs = np.asarray(bs, dtype=np.float32)
    W_out = np.asarray(W_out, dtype=np.float32)
    b_out = np.asarray(b_out, dtype=np.float32)

    ctx = input[:, :, 0].reshape(G, T, D)
    ent = input[:, :, 1].reshape(G, T, D)
    ctxT = _bf16(ctx.transpose(0, 2, 1))                  # [G, D, T]
    entT = _bf16(ent.transpose(0, 2, 1))
    ctx0 = np.ascontiguousarray(ctx[:, :, 0]).reshape(G, T, 1)  # fp32
    ws_bf = _bf16(Ws)
    w0b = _bf16(Ws[0][D:H])
    bsT = np.ascontiguousarray(bs.T)                      # [H, 6]
    bsrow = _bf16(bs.reshape(1, 6 * H))
    wout = _bf16(W_out)
    bout = np.broadcast_to(b_out.reshape(1, 1), (T, 1)).copy()

    in_maps = []
    for k in range(N_CORES):
        sl = slice(k * G_LOC, (k + 1) * G_LOC)
        in_maps.append({
            "ctxT": np.ascontiguousarray(ctxT[sl]),
            "entT": np.ascontiguousarray(entT[sl]),
            "ctx0": np.ascontiguousarray(ctx0[sl]),
            "Ws": ws_bf,
            "w0b": w0b,
            "bsT": bsT,
            "bsrow": bsrow,
            "wout": wout,
            "bout": bout,
        })
    return in_maps


def run_traced(trace=False, **inputs):
    """Returns (output [G], exec_time_ns or None)."""
    nc = _get_nc(bias_zero=bool(np.all(np.asarray(inputs["bs"]) == 0)))
    in_maps = _prep_in_maps(**inputs)
    res = run_bass_kernel_spmd(nc, in_maps, list(range(N_CORES)), trace=trace)
    out = np.concatenate([res.results[k]["out"].reshape(G_LOC)
                          for k in range(N_CORES)])
    return out, res.exec_time_ns


def kernel(**inputs) -> np.ndarray:
    out, _ = run_traced(trace=False, **inputs)
    return out


# revision 48
# speedup vs baseline: 1.2259x; 1.0399x over previous
"""Trainium2 Bass kernel for nn_MlpwithSOMModule (pairwise-concat MLP + max/mask/sum).

Reference computation (B=8, C=4, T=128, D=64, H=128, G=B*C=32):
  entity  = input[:,:,1] -> [G,T,D];  context = input[:,:,0] -> [G,T,D]
  mask    = (context[:,:,0] != 0)                         [G,T]
  x[g,i,j] = concat(context[g,i], entity[g,j])            [G,T,T,2D]
  for l in 0..5: x = tanh(x @ Ws[l] + bs[l])
  score  = (x @ W_out + b_out)[...,0]                     [G,T,T]
  out[g] = sum_i( max_j(score[g,i,j]) * mask[g,i] )       [G]

Sharding: data-parallel over G across 8 cores (4 groups/core); weights
replicated.  On-chip layout is feature-major ([128 features, pairs]) so every
MLP layer is one stationary-weight matmul.  Layer 0 uses the concat split:
  x0 = ctx_i @ W0[:D] + ent_j @ W0[D:]  ->  A[:,i] + Bb[:,j]
with A, Bb computed once per group as [128,128] matrices.

Matmul operands and activations run in bf16 (PSUM accumulation and all
bias/score/max/sum arithmetic stay fp32); the mask is computed from an fp32
slice of the context so (x != 0) is exact.
"""

import numpy as np
import ml_dtypes

import concourse.bacc as bacc
import concourse.mybir as mybir
import concourse.tile as tile
from concourse.bass_utils import run_bass_kernel_spmd

B, C, T, D = 8, 4, 128, 64
H = 2 * D          # 128
G = B * C          # 32 groups
N_CORES = 8
G_LOC = G // N_CORES   # 4 groups per core
NJ_CHUNK = 16          # j's per chunk
CHUNK = NJ_CHUNK * T   # 2048 pairs per chunk
N_CHUNKS = T // NJ_CHUNK  # 8 chunks per group

F32 = mybir.dt.float32
BF16 = mybir.dt.bfloat16
AF = mybir.ActivationFunctionType
ALU = mybir.AluOpType
AX = mybir.AxisListType

# tanh(x) ~= clip(x*E*(t^2+A1*t+B1)*(t^2+A2*t+B2), -1, 1), t=x^2 -- a deg-9
# odd minimax fit on [-3,3] (max err 5.4e-3; runs off above +1/below -1
# outside so the clip is exact there).  Evaluated in two fused custom DVE
# ops so part of the tanh load moves off the (saturated) Scalar engine.
PA1, PB1 = -21.916163674629964, 136.55212619042595
PA2, PB2 = -3.6690720803481134, 36.960418547287524
PE_ = 0.00019335252160606277

_cached_nc = {}


def _poly_layers(cc):
    """Which MLP layers of chunk cc run on the DVE poly instead of ACT tanh.
    Only early layers (the approximation error gets attenuated by later
    exact-tanh layers); 52 of the 160 (chunk, layer) instances go to the
    DVE, balancing ACT ~= DVE time.  Chunks are pipelined with a 3-stage
    stagger, so some chunk is always in its DVE phase (layers 1-2) while
    another is in its ACT phase (layers 3-5)."""
    return (1, 2) if cc % 8 < 5 else (1,)


def _register_poly_ops():
    """Register the two tanh-poly custom DVE ops (idempotent).
      op A: u = x*C2 * (sq(sq(x)) + sq(x)*C0 + C1)
      op B: y = clip(u * (sq(sq(x)) + sq(x)*C0 + C1), -1, 1), x via Src1
    """
    import concourse.dve_ops as DO
    from concourse.dve_spec import Spec, Src0, Src1, C0, C1, C2, Zero, One, \
        sq, maxx, minn, lower
    from concourse.dve_uop import DveOpSpec
    from concourse.dve_table_gen import dve_ver_for
    from concourse.dve_ops import has_src1

    if "TANH_POLY_A" in DO._SUB_OPCODE_FOR_NAME:
        return DO._POLY_OPS

    tA = sq(Src0)
    bodyA = (Src0 * C2) * (sq(tA) + tA * C0 + C1)
    tB = sq(Src1)
    bodyB = minn(maxx(Src0 * (sq(tB) + tB * C0 + C1), Zero - One), One)

    ver = dve_ver_for("TRN2")
    ops = []
    for name, body in (("TANH_POLY_A", bodyA), ("TANH_POLY_B", bodyB)):
        spec = Spec(body=body)
        row = DO._CUSTOM_DVE_ROW_BASE + len(DO.OPS)
        tmp = DveOpSpec(name=name, opcode=row, uops=lower(spec, ver=ver),
                        rd1_en=has_src1(spec))
        op = DO.DveOp(name, spec, subdim=False, uops_sha={ver: tmp.sha(ver)})
        DO.OPS.append(op)
        DO._SUB_OPCODE_FOR_NAME[name] = row
        DO.CUSTOM_DVE_SPECS[name] = spec
        ops.append(op)
    DO._POLY_OPS = tuple(ops)
    return DO._POLY_OPS


def _build_program(bias_zero):
    opA, opB = _register_poly_ops()
    nc = bacc.Bacc("TRN2", target_bir_lowering=False, debug=False,
                   num_devices=N_CORES)

    ctxT_d = nc.dram_tensor("ctxT", [G_LOC, D, T], BF16, kind="ExternalInput")
    entT_d = nc.dram_tensor("entT", [G_LOC, D, T], BF16, kind="ExternalInput")
    ctx0_d = nc.dram_tensor("ctx0", [G_LOC, T, 1], F32, kind="ExternalInput")
    ws_d = nc.dram_tensor("Ws", [6, H, H], BF16, kind="ExternalInput")
    w0b_d = nc.dram_tensor("w0b", [D, H], BF16, kind="ExternalInput")
    bsT_d = nc.dram_tensor("bsT", [H, 6], F32, kind="ExternalInput")
    bsrow_d = nc.dram_tensor("bsrow", [1, 6 * H], BF16, kind="ExternalInput")
    wout_d = nc.dram_tensor("wout", [H, 1], BF16, kind="ExternalInput")
    bout_d = nc.dram_tensor("bout", [T, 1], F32, kind="ExternalInput")
    out_d = nc.dram_tensor("out", [1, G_LOC], F32, kind="ExternalOutput")

    with tile.TileContext(nc) as tc:
        with (
            tc.tile_pool(name="consts", bufs=1) as consts,
            tc.tile_pool(name="zpool", bufs=4) as zpool,
            tc.tile_pool(name="hpool", bufs=8) as hpool,
            tc.tile_pool(name="upool", bufs=4) as upool,
            tc.tile_pool(name="small", bufs=4) as small,
            tc.tile_pool(name="psum", bufs=4, space="PSUM") as psum,
        ):
            # dummy activation first: pulls the tanh ACT_TABLE_LOAD (~2.7us)
            # off the critical path, overlapping it with setup DMAs
            scratch_sb = consts.tile([1, 1], F32)
            scratch2_sb = consts.tile([1, 1], F32)
            nc.gpsimd.memset(scratch_sb[:], 0.0)
            nc.scalar.activation(scratch2_sb[:], scratch_sb[:], AF.Tanh)

            ws_sb = consts.tile([H, 6 * H], BF16)
            w0b_sb = consts.tile([D, H], BF16)
            bsT_sb = consts.tile([H, 6], F32)
            # layer-0 prerequisites first so chunk 0 can start ASAP
            nc.sync.dma_start(ws_sb[:, 0:H], ws_d[0])
            nc.sync.dma_start(w0b_sb[:], w0b_d[:])
            nc.sync.dma_start(bsT_sb[:], bsT_d[:])
            wout_sb = consts.tile([H, 1], BF16)
            bout_sb = consts.tile([T, 1], F32)
            ones_sb = consts.tile([T, 1], F32)
            res_sb = consts.tile([1, G_LOC], F32)
            bsrow_sb = consts.tile([1, 6 * H], BF16)
            nc.sync.dma_start(bsrow_sb[:], bsrow_d[:])
            ones512_sb = consts.tile([1, 512], BF16)
            nc.vector.memset(ones512_sb[:], 1.0)

            # Per-group setup: A/Bb first-layer matrices, mask sources.
            a_sbs, bb_sbs, ctx0_sbs, rmax_sbs = [None] * G_LOC, \
                [None] * G_LOC, [None] * G_LOC, [None] * G_LOC

            def setup_group(g):
                ctxT_sb = consts.tile([D, T], BF16, tag=f"ctx{g}")
                entT_sb = consts.tile([D, T], BF16, tag=f"ent{g}")
                ctx0_sb = consts.tile([T, 1], F32, tag=f"ctx0_{g}")
                nc.sync.dma_start(ctxT_sb[:], ctxT_d[g])
                nc.sync.dma_start(entT_sb[:], entT_d[g])
                nc.sync.dma_start(ctx0_sb[:], ctx0_d[g])
                # A = (ctx @ W0_top).T : [H, T(i)];  Bb = (ent @ W0_bot).T + b0
                ps_ab = psum.tile([H, CHUNK // 2], F32, tag="mm")
                nc.tensor.matmul(ps_ab[:, 0:T], ws_sb[0:D, 0:H],
                                 ctxT_sb[:], start=True, stop=True)
                nc.tensor.matmul(ps_ab[:, 512:512 + T], w0b_sb[:],
                                 entT_sb[:], start=True, stop=True)
                a_sb = consts.tile([H, T], BF16, tag=f"a{g}")
                nc.vector.tensor_copy(a_sb[:], ps_ab[:, 0:T])
                bb_sb = consts.tile([H, T], BF16, tag=f"bb{g}")
                nc.vector.tensor_scalar_add(bb_sb[:], ps_ab[:, 512:512 + T],
                                            bsT_sb[:, 0:1])
                rmax_sb = consts.tile([T, 1], F32, tag=f"rmax{g}")
                nc.vector.memset(rmax_sb[:], -1e30)
                a_sbs[g], bb_sbs[g] = a_sb, bb_sb
                ctx0_sbs[g], rmax_sbs[g] = ctx0_sb, rmax_sb

            def build_z(cc):
                """Layer 0 for global chunk cc: z[:, jl*T+i] = A[:,i]+Bb[:,j].
                One broadcast-AP tensor_tensor on the (otherwise idle) GpSimd
                engine, keeping the Vector engine free for the tanh polys."""
                g = cc // N_CHUNKS
                c = cc % N_CHUNKS
                z_sb = zpool.tile([H, CHUNK], BF16, tag="z")
                from concourse.bass import broadcast_tensor_aps
                a_ap = a_sbs[g][:].rearrange("p (o i) -> p o i", o=1)
                b_ap = bb_sbs[g][:, c * NJ_CHUNK:(c + 1) * NJ_CHUNK] \
                    .rearrange("p (j o) -> p j o", o=1)
                a_b, b_b = broadcast_tensor_aps(a_ap, b_ap)
                z_ap = z_sb[:].rearrange("p (j i) -> p j i", j=NJ_CHUNK)
                nc.gpsimd.tensor_tensor(z_ap, a_b, b_b, op=ALU.add)
                return z_sb

            # group 0 + first z tiles go first so ACT can start ASAP;
            # the rest of the setup DMAs overlap with the first chunks
            setup_group(0)
            z_tiles = {0: build_z(0), 1: build_z(1)}
            for l in range(1, 6):
                nc.sync.dma_start(ws_sb[:, l * H:(l + 1) * H], ws_d[l])
            nc.sync.dma_start(wout_sb[:], wout_d[:])
            nc.sync.dma_start(bout_sb[:], bout_d[:])
            nc.vector.memset(ones_sb[:], 1.0)
            for g in range(1, G_LOC):
                setup_group(g)

            TOT = G_LOC * N_CHUNKS  # 32 chunks
            HC = CHUNK // 2  # 1024: psum tiles span 2 banks -> 4 slots
            h_cur = {}

            def stage_tanh1(cc):
                h_sb = hpool.tile([H, CHUNK], BF16, tag="h")
                nc.scalar.activation(h_sb[:], z_tiles.pop(cc)[:], AF.Tanh)
                h_cur[cc] = h_sb
                if cc + 2 < TOT:
                    z_tiles[cc + 2] = build_z(cc + 2)

            def stage_layer(cc, l):
                use_poly = l in _poly_layers(cc)
                need_bias_mm = use_poly and not bias_zero
                ht = h_cur[cc]
                h2_sb = hpool.tile([H, CHUNK], BF16, tag="h")
                for half in range(2):
                    ps = psum.tile([H, HC], F32, tag="mm")
                    if need_bias_mm:
                        # fold the bias in on the PE (rank-1 matmuls) since
                        # the DVE poly has no bias slot
                        for q in range(2):
                            nc.tensor.matmul(
                                ps[:, q * 512:(q + 1) * 512],
                                bsrow_sb[0:1, l * H:(l + 1) * H],
                                ones512_sb[:], start=True, stop=False)
                    for q in range(2):
                        o = half * HC + q * 512
                        nc.tensor.matmul(
                            ps[:, q * 512:(q + 1) * 512],
                            ws_sb[:, l * H:(l + 1) * H],
                            ht[:, o:o + 512],
                            start=not need_bias_mm, stop=True)
                    hsl = h2_sb[:, half * HC:(half + 1) * HC]
                    if use_poly:
                        u_sb = upool.tile([H, HC], F32, tag="u")
                        nc.vector._custom_dve(opA, out=u_sb[:], in0=ps[:],
                                              s0=PA1, s1=PB1, imm2=PE_)
                        nc.vector._custom_dve(opB, out=hsl, in0=u_sb[:],
                                              in1=ps[:], s0=PA2, s1=PB2)
                    else:
                        nc.scalar.activation(hsl, ps[:], AF.Tanh,
                                             bias=bsT_sb[:, l:l + 1])
                h_cur[cc] = h2_sb

            def stage_score(cc):
                g = cc // N_CHUNKS
                ht = h_cur.pop(cc)
                # final layer, transposed: score col [T(i), 1] per j via
                # stationary h-block x moving W_out
                sc_ps = psum.tile([H, HC], F32, tag="mm")
                for jl in range(NJ_CHUNK):
                    nc.tensor.matmul(
                        sc_ps[:, jl:jl + 1],
                        ht[:, jl * T:(jl + 1) * T],
                        wout_sb[:], start=True, stop=True)
                # max over the 16 j's, fold into running max (both [T,1])
                tmp_sb = small.tile([T, 1], F32, tag="tmp")
                nc.vector.tensor_reduce(tmp_sb[:], sc_ps[:, 0:NJ_CHUNK],
                                        axis=AX.X, op=ALU.max)
                nc.vector.tensor_max(rmax_sbs[g][:], rmax_sbs[g][:],
                                     tmp_sb[:])

            # Layer-staggered software pipeline: chunk c runs stage t
            # (0=tanh1, 1..5=MLP layers, 6=score) at position 3c+t, so ~3
            # chunks are in flight at complementary ACT/DVE phases.
            STAGGER = 3
            events = sorted(
                (c * STAGGER + t, c, t)
                for c in range(TOT) for t in range(7))
            for _pos, cc, t in events:
                if t == 0:
                    stage_tanh1(cc)
                elif t <= 5:
                    stage_layer(cc, t)
                else:
                    stage_score(cc)

            for g in range(G_LOC):
                # mask = (ctx[:,0] != 0); out = sum_i(mask*(rmax+b_out))
                mask_sb = small.tile([T, 1], F32, tag="mask")
                nc.vector.tensor_scalar(mask_sb[:], ctx0_sbs[g][:], 0.0, None,
                                        op0=ALU.not_equal)
                rb_sb = small.tile([T, 1], F32, tag="rb")
                nc.vector.tensor_scalar_add(rb_sb[:], rmax_sbs[g][:],
                                            bout_sb[:, 0:1])
                mm_sb = small.tile([T, 1], F32, tag="mmul")
                nc.vector.tensor_mul(mm_sb[:], rb_sb[:], mask_sb[:])
                # partition-axis sum via ones-matmul: [1,1] = mm.T @ ones
                sum_ps = psum.tile([H, CHUNK // 2], F32, tag="mm")
                nc.tensor.matmul(sum_ps[0:1, 0:1], mm_sb[:], ones_sb[:],
                                 start=True, stop=True)
                nc.vector.tensor_copy(res_sb[0:1, g:g + 1], sum_ps[0:1, 0:1])

            nc.sync.dma_start(out_d[:], res_sb[:])

    nc.compile()
    return nc


def _get_nc(bias_zero):
    if bias_zero not in _cached_nc:
        _cached_nc[bias_zero] = _build_program(bias_zero)
    return _cached_nc[bias_zero]


def _bf16(a):
    return np.ascontiguousarray(a.astype(ml_dtypes.bfloat16))


def _prep_in_maps(input, Ws, bs, W_out, b_out):
    input = np.ascontiguousarray(np.asarray(input, dtype=np.float32))
    Ws = np.asarray(Ws, dtype=np.float32)
    bs = np.asarray(bs, dtype=np.float32)
    W_out = np.asarray(W_out, dtype=np.float32)
    b_out = np.asarray(b_out, dtype=np.float32)

    ctx = input[:, :, 0].reshape(G, T, D)
    ent = input[:, :, 1].reshape(G, T, D)
    ctxT = _bf16(ctx.transpose(0, 2, 1))                  # [G, D, T]
    entT = _bf16(ent.transpose(0, 2, 1))
    ctx0 = np.ascontiguousarray(ctx[:, :, 0]).reshape(G, T, 1)  # fp32
    ws_bf = _bf16(Ws)
    w0b = _bf16(Ws[0][D:H])
    bsT = np.ascontiguousarray(bs.T)                      # [H, 6]
    bsrow = _bf16(bs.reshape(1, 6 * H))
    wout = _bf16(W_out)
    bout = np.broadcast_to(b_out.reshape(1, 1), (T, 1)).copy()

    in_maps = []
    for k in range(N_CORES):
        sl = slice(k * G_LOC, (k + 1) * G_LOC)
        in_maps.append({
            "ctxT": np.ascontiguousarray(ctxT[sl]),
            "entT": np.ascontiguousarray(entT[sl]),
            "ctx0": np.ascontiguousarray(ctx0[sl]),
            "Ws": ws_bf,
            "w0b": w0b,
            "bsT": bsT,
            "bsrow": bsrow,
            "wout": wout,
            "bout": bout,
        })
    return in_maps


def run_traced(trace=False, **inputs):
    """Returns (output [G], exec_time_ns or None)."""
    nc = _get_nc(bias_zero=bool(np.all(np.asarray(inputs["bs"]) == 0)))
    in_maps = _prep_in_maps(**inputs)
    res = run_bass_kernel_spmd(nc, in_maps, list(range(N_CORES)), trace=trace)
    out = np.concatenate([res.results[k]["out"].reshape(G_LOC)
                          for k in range(N_CORES)])
    return out, res.exec_time_ns


def kernel(**inputs) -> np.ndarray:
    out, _ = run_traced(trace=False, **inputs)
    return out


# revision 49
# speedup vs baseline: 1.4775x; 1.2053x over previous
"""Trainium2 Bass kernel for nn_MlpwithSOMModule (pairwise-concat MLP + max/mask/sum).

Reference computation (B=8, C=4, T=128, D=64, H=128, G=B*C=32):
  entity  = input[:,:,1] -> [G,T,D];  context = input[:,:,0] -> [G,T,D]
  mask    = (context[:,:,0] != 0)                         [G,T]
  x[g,i,j] = concat(context[g,i], entity[g,j])            [G,T,T,2D]
  for l in 0..5: x = tanh(x @ Ws[l] + bs[l])
  score  = (x @ W_out + b_out)[...,0]                     [G,T,T]
  out[g] = sum_i( max_j(score[g,i,j]) * mask[g,i] )       [G]

Sharding: data-parallel over G across 8 cores (4 groups/core); weights
replicated.  On-chip layout is feature-major ([128 features, pairs]) so every
MLP layer is one stationary-weight matmul.  Layer 0 uses the concat split:
  x0 = ctx_i @ W0[:D] + ent_j @ W0[D:]  ->  A[:,i] + Bb[:,j]
with A, Bb computed once per group as [128,128] matrices.

Matmul operands and activations run in bf16 (PSUM accumulation and all
bias/score/max/sum arithmetic stay fp32); the mask is computed from an fp32
slice of the context so (x != 0) is exact.
"""

import numpy as np
import ml_dtypes

import concourse.bacc as bacc
import concourse.mybir as mybir
import concourse.tile as tile
from concourse.bass_utils import run_bass_kernel_spmd

B, C, T, D = 8, 4, 128, 64
H = 2 * D          # 128
G = B * C          # 32 groups
N_CORES = 8
G_LOC = G // N_CORES   # 4 groups per core
NJ_CHUNK = 16          # j's per chunk
CHUNK = NJ_CHUNK * T   # 2048 pairs per chunk
N_CHUNKS = T // NJ_CHUNK  # 8 chunks per group

F32 = mybir.dt.float32
BF16 = mybir.dt.bfloat16
AF = mybir.ActivationFunctionType
ALU = mybir.AluOpType
AX = mybir.AxisListType

# tanh(x) ~= clip(x*E*(t^2+A1*t+B1)*(t^2+A2*t+B2), -1, 1), t=x^2 -- a deg-9
# odd minimax fit on [-3,3] (max err 5.4e-3; runs off above +1/below -1
# outside so the clip is exact there).  Evaluated in two fused custom DVE
# ops so part of the tanh load moves off the (saturated) Scalar engine.
PA1, PB1 = -21.916163674629964, 136.55212619042595
PA2, PB2 = -3.6690720803481134, 36.960418547287524
PE_ = 0.00019335252160606277

_cached_nc = {}


def _poly_layers(cc):
    """Which MLP layers of chunk cc run on the DVE poly instead of ACT tanh.
    Only early layers (the approximation error gets attenuated by later
    exact-tanh layers); 52 of the 160 (chunk, layer) instances go to the
    DVE, balancing ACT ~= DVE time.  Chunks are pipelined with a 3-stage
    stagger, so some chunk is always in its DVE phase (layers 1-2) while
    another is in its ACT phase (layers 3-5)."""
    return (1, 2) if cc % 8 < 5 else (1,)


def _register_poly_ops():
    """Register the two tanh-poly custom DVE ops (idempotent).
      op A: u = x*C2 * (sq(sq(x)) + sq(x)*C0 + C1)
      op B: y = clip(u * (sq(sq(x)) + sq(x)*C0 + C1), -1, 1), x via Src1
    """
    import concourse.dve_ops as DO
    from concourse.dve_spec import Spec, Src0, Src1, C0, C1, C2, Zero, One, \
        sq, maxx, minn, lower
    from concourse.dve_uop import DveOpSpec
    from concourse.dve_table_gen import dve_ver_for
    from concourse.dve_ops import has_src1

    if "TANH_POLY_A" in DO._SUB_OPCODE_FOR_NAME:
        return DO._POLY_OPS

    tA = sq(Src0)
    bodyA = (Src0 * C2) * (sq(tA) + tA * C0 + C1)
    tB = sq(Src1)
    bodyB = minn(maxx(Src0 * (sq(tB) + tB * C0 + C1), Zero - One), One)

    ver = dve_ver_for("TRN2")
    ops = []
    for name, body in (("TANH_POLY_A", bodyA), ("TANH_POLY_B", bodyB)):
        spec = Spec(body=body)
        row = DO._CUSTOM_DVE_ROW_BASE + len(DO.OPS)
        tmp = DveOpSpec(name=name, opcode=row, uops=lower(spec, ver=ver),
                        rd1_en=has_src1(spec))
        op = DO.DveOp(name, spec, subdim=False, uops_sha={ver: tmp.sha(ver)})
        DO.OPS.append(op)
        DO._SUB_OPCODE_FOR_NAME[name] = row
        DO.CUSTOM_DVE_SPECS[name] = spec
        ops.append(op)
    DO._POLY_OPS = tuple(ops)
    return DO._POLY_OPS


def _build_program(bias_zero):
    opA, opB = _register_poly_ops()
    nc = bacc.Bacc("TRN2", target_bir_lowering=False, debug=False,
                   num_devices=N_CORES)

    ctxT_d = nc.dram_tensor("ctxT", [G_LOC, D, T], BF16, kind="ExternalInput")
    entT_d = nc.dram_tensor("entT", [G_LOC, D, T], BF16, kind="ExternalInput")
    ctx0_d = nc.dram_tensor("ctx0", [G_LOC, T, 1], F32, kind="ExternalInput")
    ws_d = nc.dram_tensor("Ws", [6, H, H], BF16, kind="ExternalInput")
    w0b_d = nc.dram_tensor("w0b", [D, H], BF16, kind="ExternalInput")
    bsT_d = nc.dram_tensor("bsT", [H, 6], F32, kind="ExternalInput")
    bsrow_d = nc.dram_tensor("bsrow", [1, 6 * H], BF16, kind="ExternalInput")
    wout_d = nc.dram_tensor("wout", [H, 1], BF16, kind="ExternalInput")
    bout_d = nc.dram_tensor("bout", [T, 1], F32, kind="ExternalInput")
    out_d = nc.dram_tensor("out", [1, G_LOC], F32, kind="ExternalOutput")

    with tile.TileContext(nc) as tc:
        with (
            tc.tile_pool(name="consts", bufs=1) as consts,
            tc.tile_pool(name="zpool", bufs=6) as zpool,
            tc.tile_pool(name="hpool", bufs=8) as hpool,
            tc.tile_pool(name="upool", bufs=6) as upool,
            tc.tile_pool(name="small", bufs=4) as small,
            tc.tile_pool(name="psum", bufs=4, space="PSUM") as psum,
        ):
            # dummy activation first: pulls the tanh ACT_TABLE_LOAD (~2.7us)
            # off the critical path, overlapping it with setup DMAs
            scratch_sb = consts.tile([1, 1], F32)
            scratch2_sb = consts.tile([1, 1], F32)
            nc.gpsimd.memset(scratch_sb[:], 0.0)
            nc.scalar.activation(scratch2_sb[:], scratch_sb[:], AF.Tanh)

            ws_sb = consts.tile([H, 6 * H], BF16)
            w0b_sb = consts.tile([D, H], BF16)
            bsT_sb = consts.tile([H, 6], F32)
            # layer-0 prerequisites first so chunk 0 can start ASAP
            nc.sync.dma_start(ws_sb[:, 0:H], ws_d[0])
            nc.sync.dma_start(w0b_sb[:], w0b_d[:])
            nc.sync.dma_start(bsT_sb[:], bsT_d[:])
            wout_sb = consts.tile([H, 1], BF16)
            bout_sb = consts.tile([T, 1], F32)
            ones_sb = consts.tile([T, 1], F32)
            res_sb = consts.tile([1, G_LOC], F32)
            bsrow_sb = consts.tile([1, 6 * H], BF16)
            nc.sync.dma_start(bsrow_sb[:], bsrow_d[:])
            ones512_sb = consts.tile([1, 512], BF16)
            nc.vector.memset(ones512_sb[:], 1.0)

            # Per-group setup: A/Bb first-layer matrices, mask sources.
            a_sbs, bb_sbs, ctx0_sbs, rmax_sbs = [None] * G_LOC, \
                [None] * G_LOC, [None] * G_LOC, [None] * G_LOC

            def setup_group(g):
                ctxT_sb = consts.tile([D, T], BF16, tag=f"ctx{g}")
                entT_sb = consts.tile([D, T], BF16, tag=f"ent{g}")
                ctx0_sb = consts.tile([T, 1], F32, tag=f"ctx0_{g}")
                nc.sync.dma_start(ctxT_sb[:], ctxT_d[g])
                nc.sync.dma_start(entT_sb[:], entT_d[g])
                nc.sync.dma_start(ctx0_sb[:], ctx0_d[g])
                # A = (ctx @ W0_top).T : [H, T(i)];  Bb = (ent @ W0_bot).T + b0
                ps_ab = psum.tile([H, CHUNK // 2], F32, tag="mm")
                nc.tensor.matmul(ps_ab[:, 0:T], ws_sb[0:D, 0:H],
                                 ctxT_sb[:], start=True, stop=True)
                nc.tensor.matmul(ps_ab[:, 512:512 + T], w0b_sb[:],
                                 entT_sb[:], start=True, stop=True)
                a_sb = consts.tile([H, T], BF16, tag=f"a{g}")
                nc.vector.tensor_copy(a_sb[:], ps_ab[:, 0:T])
                bb_sb = consts.tile([H, T], BF16, tag=f"bb{g}")
                nc.vector.tensor_scalar_add(bb_sb[:], ps_ab[:, 512:512 + T],
                                            bsT_sb[:, 0:1])
                rmax_sb = consts.tile([T, 1], F32, tag=f"rmax{g}")
                nc.vector.memset(rmax_sb[:], -1e30)
                a_sbs[g], bb_sbs[g] = a_sb, bb_sb
                ctx0_sbs[g], rmax_sbs[g] = ctx0_sb, rmax_sb

            def build_z(cc):
                """Layer 0 for global chunk cc: z[:, jl*T+i] = A[:,i]+Bb[:,j].
                One broadcast-AP tensor_tensor on the (otherwise idle) GpSimd
                engine, keeping the Vector engine free for the tanh polys."""
                g = cc // N_CHUNKS
                c = cc % N_CHUNKS
                z_sb = zpool.tile([H, CHUNK], BF16, tag="z")
                from concourse.bass import broadcast_tensor_aps
                a_ap = a_sbs[g][:].rearrange("p (o i) -> p o i", o=1)
                b_ap = bb_sbs[g][:, c * NJ_CHUNK:(c + 1) * NJ_CHUNK] \
                    .rearrange("p (j o) -> p j o", o=1)
                a_b, b_b = broadcast_tensor_aps(a_ap, b_ap)
                z_ap = z_sb[:].rearrange("p (j i) -> p j i", j=NJ_CHUNK)
                nc.gpsimd.tensor_tensor(z_ap, a_b, b_b, op=ALU.add)
                return z_sb

            # group 0 + first z tiles go first so ACT can start ASAP;
            # the rest of the setup DMAs overlap with the first chunks
            setup_group(0)
            z_tiles = {0: build_z(0), 1: build_z(1)}
            for l in range(1, 6):
                nc.sync.dma_start(ws_sb[:, l * H:(l + 1) * H], ws_d[l])
            nc.sync.dma_start(wout_sb[:], wout_d[:])
            nc.sync.dma_start(bout_sb[:], bout_d[:])
            nc.vector.memset(ones_sb[:], 1.0)
            for g in range(1, G_LOC):
                setup_group(g)

            TOT = G_LOC * N_CHUNKS  # 32 chunks
            HC = CHUNK // 2  # 1024: psum tiles span 2 banks -> 4 slots
            h_cur = {}

            def stage_tanh1(cc):
                h_sb = hpool.tile([H, CHUNK], BF16, tag="h")
                nc.scalar.activation(h_sb[:], z_tiles.pop(cc)[:], AF.Tanh)
                h_cur[cc] = h_sb
                if cc + 2 < TOT:
                    z_tiles[cc + 2] = build_z(cc + 2)

            def stage_layer(cc, l):
                use_poly = l in _poly_layers(cc)
                need_bias_mm = use_poly and not bias_zero
                ht = h_cur[cc]
                h2_sb = hpool.tile([H, CHUNK], BF16, tag="h")
                for half in range(2):
                    ps = psum.tile([H, HC], F32, tag="mm")
                    if need_bias_mm:
                        # fold the bias in on the PE (rank-1 matmuls) since
                        # the DVE poly has no bias slot
                        for q in range(2):
                            nc.tensor.matmul(
                                ps[:, q * 512:(q + 1) * 512],
                                bsrow_sb[0:1, l * H:(l + 1) * H],
                                ones512_sb[:], start=True, stop=False)
                    for q in range(2):
                        o = half * HC + q * 512
                        nc.tensor.matmul(
                            ps[:, q * 512:(q + 1) * 512],
                            ws_sb[:, l * H:(l + 1) * H],
                            ht[:, o:o + 512],
                            start=not need_bias_mm, stop=True)
                    hsl = h2_sb[:, half * HC:(half + 1) * HC]
                    if use_poly:
                        u_sb = upool.tile([H, HC], F32, tag="u")
                        nc.vector._custom_dve(opA, out=u_sb[:], in0=ps[:],
                                              s0=PA1, s1=PB1, imm2=PE_)
                        nc.vector._custom_dve(opB, out=hsl, in0=u_sb[:],
                                              in1=ps[:], s0=PA2, s1=PB2)
                    else:
                        nc.scalar.activation(hsl, ps[:], AF.Tanh,
                                             bias=bsT_sb[:, l:l + 1])
                h_cur[cc] = h2_sb

            def stage_score(cc):
                g = cc // N_CHUNKS
                ht = h_cur.pop(cc)
                # final layer, transposed: score col [T(i), 1] per j via
                # stationary h-block x moving W_out
                sc_ps = psum.tile([H, HC], F32, tag="mm")
                for jl in range(NJ_CHUNK):
                    nc.tensor.matmul(
                        sc_ps[:, jl:jl + 1],
                        ht[:, jl * T:(jl + 1) * T],
                        wout_sb[:], start=True, stop=True)
                # max over the 16 j's, fold into running max (both [T,1])
                tmp_sb = small.tile([T, 1], F32, tag="tmp")
                nc.vector.tensor_reduce(tmp_sb[:], sc_ps[:, 0:NJ_CHUNK],
                                        axis=AX.X, op=ALU.max)
                nc.vector.tensor_max(rmax_sbs[g][:], rmax_sbs[g][:],
                                     tmp_sb[:])

            # Layer-staggered software pipeline: chunk c runs stage t
            # (0=tanh1, 1..5=MLP layers, 6=score) at position 3c+t, so ~3
            # chunks are in flight at complementary ACT/DVE phases.
            STAGGER = 2
            events = sorted(
                (c * STAGGER + t, c, t)
                for c in range(TOT) for t in range(7))
            for _pos, cc, t in events:
                if t == 0:
                    stage_tanh1(cc)
                elif t <= 5:
                    stage_layer(cc, t)
                else:
                    stage_score(cc)

            for g in range(G_LOC):
                # mask = (ctx[:,0] != 0); out = sum_i(mask*(rmax+b_out))
                mask_sb = small.tile([T, 1], F32, tag="mask")
                nc.vector.tensor_scalar(mask_sb[:], ctx0_sbs[g][:], 0.0, None,
                                        op0=ALU.not_equal)
                rb_sb = small.tile([T, 1], F32, tag="rb")
                nc.vector.tensor_scalar_add(rb_sb[:], rmax_sbs[g][:],
                                            bout_sb[:, 0:1])
                mm_sb = small.tile([T, 1], F32, tag="mmul")
                nc.vector.tensor_mul(mm_sb[:], rb_sb[:], mask_sb[:])
                # partition-axis sum via ones-matmul: [1,1] = mm.T @ ones
                sum_ps = psum.tile([H, CHUNK // 2], F32, tag="mm")
                nc.tensor.matmul(sum_ps[0:1, 0:1], mm_sb[:], ones_sb[:],
                                 start=True, stop=True)
                nc.vector.tensor_copy(res_sb[0:1, g:g + 1], sum_ps[0:1, 0:1])

            nc.sync.dma_start(out_d[:], res_sb[:])

    nc.compile()
    return nc


def _get_nc(bias_zero):
    if bias_zero not in _cached_nc:
        _cached_nc[bias_zero] = _build_program(bias_zero)
    return _cached_nc[bias_zero]


def _bf16(a):
    return np.ascontiguousarray(a.astype(ml_dtypes.bfloat16))


def _prep_in_maps(input, Ws, bs, W_out, b_out):
    input = np.ascontiguousarray(np.asarray(input, dtype=np.float32))
    Ws = np.asarray(Ws, dtype=np.float32)
    bs = np.asarray(bs, dtype=np.float32)
    W_out = np.asarray(W_out, dtype=np.float32)
    b_out = np.asarray(b_out, dtype=np.float32)

    ctx = input[:, :, 0].reshape(G, T, D)
    ent = input[:, :, 1].reshape(G, T, D)
    ctxT = _bf16(ctx.transpose(0, 2, 1))                  # [G, D, T]
    entT = _bf16(ent.transpose(0, 2, 1))
    ctx0 = np.ascontiguousarray(ctx[:, :, 0]).reshape(G, T, 1)  # fp32
    ws_bf = _bf16(Ws)
    w0b = _bf16(Ws[0][D:H])
    bsT = np.ascontiguousarray(bs.T)                      # [H, 6]
    bsrow = _bf16(bs.reshape(1, 6 * H))
    wout = _bf16(W_out)
    bout = np.broadcast_to(b_out.reshape(1, 1), (T, 1)).copy()

    in_maps = []
    for k in range(N_CORES):
        sl = slice(k * G_LOC, (k + 1) * G_LOC)
        in_maps.append({
            "ctxT": np.ascontiguousarray(ctxT[sl]),
            "entT": np.ascontiguousarray(entT[sl]),
            "ctx0": np.ascontiguousarray(ctx0[sl]),
            "Ws": ws_bf,
            "w0b": w0b,
            "bsT": bsT,
            "bsrow": bsrow,
            "wout": wout,
            "bout": bout,
        })
    return in_maps


def run_traced(trace=False, **inputs):
    """Returns (output [G], exec_time_ns or None)."""
    nc = _get_nc(bias_zero=bool(np.all(np.asarray(inputs["bs"]) == 0)))
    in_maps = _prep_in_maps(**inputs)
    res = run_bass_kernel_spmd(nc, in_maps, list(range(N_CORES)), trace=trace)
    out = np.concatenate([res.results[k]["out"].reshape(G_LOC)
                          for k in range(N_CORES)])
    return out, res.exec_time_ns


def kernel(**inputs) -> np.ndarray:
    out, _ = run_traced(trace=False, **inputs)
    return out
